# revision 6
# baseline (speedup 1.0000x reference)
"""Butterfly multiply (n=4096, 12 stages, increasing stride) on 8 Trainium2
NeuronCores.

Math: the 12 butterfly stages factor into two dense matmul passes:
stages 0..6 (strides 1..64) compose into 32 dense 128x128 matrices A_o acting
within 128-aligned blocks; stages 7..11 (strides 128..2048) compose into 128
dense 32x32 matrices C_i acting across blocks at fixed within-block index.
Both are composed on the host from the tiny twiddle input.

Device pipeline per core (batch shard 1024 rows, fp16 compute):
  load x^T  ->  pass A (AT_o stationary)  ->  PSUM copy -> y1
  -> permute DMA  z[(a,o), (tau,b)] = y_mid[b, o*128+4tau+a]  (SBUF->SBUF)
  -> pass B (R_tau stationary, z moving)  -> psB [(o',a)-part, b]  (n-major)
  -> PSUM copy -> outT -> store n-major; host unscrambles columns.

The permute is SBUF read-port limited: each per-tau DMA reads 4 source
partitions.  Since pass A's output partition is a free column permutation of
the host-built AT matrices, y1 row (tau,a) is placed at partition
16*m(tau) + 4a + r(tau) so the 4 reads of one tau-DMA hit 4 DIFFERENT SBUF
AXI ports, and the tau->(m,r) map is built so the sync ring's taus (j in
{0,1} of each 4-tau wave) use even ports while the scalar ring's (j in {2,3})
use odd ports -- the two rings never collide on read ports.

DMA layout keeps descriptor runs big: xt dram [p, o, b] (load runs 32 KB),
y dram [v, tau, b] (store runs 8 KB), permute runs 2 KB (full 1024 batch).

Sharding: batch 8192 split across 8 cores (data parallel), twiddle-derived
matrices replicated.
"""

import os
import numpy as np

LOG_N = 12
N = 4096
BATCH = 8192
N_CORES = 8
B_CORE = BATCH // N_CORES  # 1024 rows per core

COMPUTE = os.environ.get("BUTTERFLY_COMPUTE", "fp16")
SPREAD = os.environ.get("BUTTERFLY_SPREAD", "1") == "1"
STORE_LAG = int(os.environ.get("BUTTERFLY_STORE_LAG", "2"))
PERM_GP = int(os.environ.get("BUTTERFLY_PERM_GP", "4"))
OG = 4            # o's per load DMA group
SUB = 512         # matmul / PSUM-copy column granularity

def _tau_h(tau):
    """Base partition + stride of tau's 4 source rows (at h, h+st, h+2st, h+3st).
    Stride 8 within a 32-partition block spreads the 4 reads across 4 SBUF AXI
    ports under both plausible port wirings (p div 8, and the quad-interleave)."""
    if not SPREAD:
        return 4 * tau, 1  # contiguous quad, stride 1
    w, j = tau >> 2, tau & 3
    sloc = (w + 4 * (j & 1)) % 8
    return 32 * j + sloc, 8


def _i_of_p():
    """i_of_p[p] = which y_mid row (i' = 4 tau + a) pass A must emit at
    partition p.  Inverse of the placement in _tau_h."""
    if not SPREAD:
        return np.arange(128)
    iop = np.empty(128, dtype=np.int64)
    for p in range(128):
        j, rem = p >> 5, p & 31
        a, sloc = rem >> 3, rem & 7
        w = (sloc - 4 * (j & 1)) % 8
        iop[p] = 4 * (4 * w + j) + a
    assert len(set(iop.tolist())) == 128
    return iop


def _compose_matrices(twiddle):
    """Compose stages 0..6 -> A (32,128,128) and stages 7..11 -> C (128,32,32),
    in float64."""
    tw = np.asarray(twiddle)[0, 0].astype(np.float64)  # (12, 2048, 2, 2)

    A = np.zeros((32, 128, 128))
    A[:, np.arange(128), np.arange(128)] = 1.0
    for idx in range(7):
        s = 1 << idx
        Ar = A.reshape(32, 128 // (2 * s), 2, s, 128)  # (o, dl, k, j, i_in)
        o = np.arange(32)[:, None, None]
        dl = np.arange(128 // (2 * s))[None, :, None]
        j = np.arange(s)[None, None, :]
        m = (o * (64 // s) + dl) * s + j
        t = tw[idx, m]  # (32, dl, j, 2, 2)
        x0, x1 = Ar[:, :, 0], Ar[:, :, 1]
        new0 = t[..., 0, 0:1] * x0 + t[..., 0, 1:2] * x1
        new1 = t[..., 1, 0:1] * x0 + t[..., 1, 1:2] * x1
        A = np.stack([new0, new1], axis=2).reshape(32, 128, 128)

    C = np.zeros((128, 32, 32))
    C[:, np.arange(32), np.arange(32)] = 1.0
    for idx in range(7, 12):
        s = 1 << idx
        sp = s // 128
        Cr = C.reshape(128, 32 // (2 * sp), 2, sp, 32)  # (i, dl, k, ol, o_in)
        i = np.arange(128)[None, None, :]
        dl = np.arange(32 // (2 * sp))[:, None, None]
        ol = np.arange(sp)[None, :, None]
        m = dl * (128 * sp) + 128 * ol + i  # (dl, ol, i)
        t = np.moveaxis(tw[idx, m], 2, 0)  # (i, dl, ol, 2, 2)
        x0, x1 = Cr[:, :, 0], Cr[:, :, 1]
        new0 = t[..., 0, 0:1] * x0 + t[..., 0, 1:2] * x1
        new1 = t[..., 1, 0:1] * x0 + t[..., 1, 1:2] * x1
        C = np.stack([new0, new1], axis=2).reshape(128, 32, 32)

    # AT'[o][k, p] = A_o[i_of_p(p), k]   (sigma-permuted lhsT for pass A)
    AT = np.ascontiguousarray(np.transpose(A[:, _i_of_p(), :], (0, 2, 1)))
    # R[tau][u=(a*32+o_in), v=(o_out*4+a)] = C[4*tau+a][o_out, o_in]
    R = np.zeros((32, 128, 128))
    for tau in range(32):
        for a in range(4):
            R[tau, a * 32:(a + 1) * 32, a::4] = C[4 * tau + a].T
    return AT, R


def _build_program(np_dt, mybir_dt):
    """Trace + compile the per-core Bass program. Returns nc."""
    import concourse.bacc as bacc
    import concourse.tile as tile
    import concourse.mybir as mybir
    from contextlib import ExitStack

    f32 = mybir.dt.float32
    dt = mybir_dt
    B = B_CORE

    nc = bacc.Bacc(
        "TRN2",
        target_bir_lowering=False,
        debug=False,
        enable_asserts=False,
        num_devices=1,
    )
    # x shipped pre-transposed, partition-major: xt[p, o, b] = x[b, o*128+p]
    x_ap = nc.dram_tensor("xt", (128, 32, B), dt, kind="ExternalInput").ap()
    at_ap = nc.dram_tensor("AT", (128, 32 * 128), dt, kind="ExternalInput").ap()
    r_ap = nc.dram_tensor("R", (128, 32 * 128), dt, kind="ExternalInput").ap()
    # n-major output: y[v, tau, b]; host maps (tau, v) -> n
    y_ap = nc.dram_tensor("y", (128, 32, B), dt, kind="ExternalOutput").ap()

    with tile.TileContext(nc) as tc, ExitStack() as ctx:
        wpool = ctx.enter_context(tc.tile_pool(name="weights", bufs=1))
        xT_pool = ctx.enter_context(tc.tile_pool(name="xT", bufs=1))
        z_pool = ctx.enter_context(tc.tile_pool(name="z", bufs=1))
        out_pool = ctx.enter_context(tc.tile_pool(name="outT", bufs=2))
        psA_pool = ctx.enter_context(tc.tile_pool(name="psA", bufs=2, space="PSUM"))
        psB_pool = ctx.enter_context(tc.tile_pool(name="psB", bufs=2, space="PSUM"))

        ATw = wpool.tile([128, 32 * 128], dt, tag="ATw")
        Rw = wpool.tile([128, 32 * 128], dt, tag="Rw")
        nc.sync.dma_start(ATw[:], at_ap)
        nc.scalar.dma_start(Rw[:], r_ap)

        xT = xT_pool.tile([128, 32 * B], dt, tag="xT")
        y1 = xT  # pass A writes back over the x block it just consumed
        z = z_pool.tile([128, 32 * B], dt, tag="z")

        # 1. loads on sync ring (runs 32 KB -> big descriptors)
        for og in range(32 // OG):
            nc.sync.dma_start(
                xT[:, og * OG * B:(og + 1) * OG * B],
                x_ap[:, og * OG:(og + 1) * OG, :],
            )

        # 2. pass A; copies rotate vector/gpsimd/scalar (all idle here)
        def _copy(eng, dst, src):
            (eng.copy if eng is nc.scalar else eng.tensor_copy)(dst, src)

        cps = [nc.vector, nc.scalar]
        for o in range(32):
            psA = psA_pool.tile([128, B], f32, tag="psA")
            for ss in range(B // SUB):
                nc.tensor.matmul(
                    psA[:, ss * SUB:(ss + 1) * SUB],
                    ATw[:, o * 128:(o + 1) * 128],
                    xT[:, o * B + ss * SUB:o * B + (ss + 1) * SUB],
                    start=True,
                    stop=True,
                )
            _copy(cps[o % 2], y1[:, o * B:(o + 1) * B], psA[:])

        # 3+4. permute waves + pass B + stores, wave = 4 consecutive taus.
        #   perm j in {0,1} -> sync ring (even read ports)
        #   perm j in {2,3} -> scalar ring (odd read ports)
        #   stores -> scalar ring, interleaved STORE_LAG waves behind perm
        out_tiles = {}

        def emit_perm(w):
            for j in range(4):
                tau = 4 * w + j
                h, st = _tau_h(tau)
                if PERM_GP and w % (8 // max(PERM_GP // 2, 1)) == 0 and j in (1, 3):
                    eng = nc.gpsimd
                else:
                    eng = nc.sync if j < 2 else nc.scalar
                eng.dma_start(
                    z[:, tau * B:(tau + 1) * B],
                    y1[h:h + 3 * st + 1:st, :].rearrange("a (o b) -> a o b", b=B),
                )

        def emit_passB(w):
            outT = out_pool.tile([128, 4 * B], dt, tag="outT")
            out_tiles[w] = outT
            cpb = [nc.vector, nc.scalar]
            for tt in range(4):
                tau = 4 * w + tt
                psB = psB_pool.tile([128, B], f32, tag="psB")
                for ss in range(B // SUB):
                    nc.tensor.matmul(
                        psB[:, ss * SUB:(ss + 1) * SUB],
                        Rw[:, tau * 128:(tau + 1) * 128],
                        z[:, tau * B + ss * SUB:tau * B + (ss + 1) * SUB],
                        start=True,
                        stop=True,
                    )
                _copy(cpb[tt % 2], outT[:, tt * B:(tt + 1) * B], psB[:])

        def emit_store(w):
            nc.scalar.dma_start(
                y_ap[:, 4 * w:4 * (w + 1), :], out_tiles.pop(w)[:]
            )

        for w in range(8):
            emit_perm(w)
            emit_passB(w)
            if w >= STORE_LAG:
                emit_store(w - STORE_LAG)
        for w in range(8 - STORE_LAG, 8):
            emit_store(w)

    nc.compile()
    return nc


_CACHE = {}


def _get_program():
    import concourse.mybir as mybir

    key = COMPUTE
    if key not in _CACHE:
        if COMPUTE == "fp16":
            _CACHE[key] = (_build_program(np.float16, mybir.dt.float16), np.float16)
        else:
            _CACHE[key] = (_build_program(np.float32, mybir.dt.float32), np.float32)
    return _CACHE[key]


def _col_gather_index():
    """c_of_n[n] = v(n)*32 + tau(n) into the (v,tau)-flattened device output."""
    n = np.arange(N)
    i = n % 128
    o_out = n // 128
    tau = i // 4
    a = i % 4
    v = o_out * 4 + a
    return v * 32 + tau


def run(x, twiddle, trace=False, trace_kwargs=None):
    """Run the butterfly kernel on 8 cores. Returns (out, BassKernelResults)."""
    from concourse.bass_utils import run_bass_kernel_spmd

    nc, np_dt = _get_program()

    AT, R = _compose_matrices(twiddle)
    ATd = np.ascontiguousarray(AT.transpose(1, 0, 2).reshape(128, 32 * 128)).astype(np_dt)
    Rd = np.ascontiguousarray(R.transpose(1, 0, 2).reshape(128, 32 * 128)).astype(np_dt)

    x = np.asarray(x)
    in_dtype = x.dtype
    xd = x.astype(np_dt)

    in_maps = []
    for c in range(N_CORES):
        shard = xd[c * B_CORE:(c + 1) * B_CORE]
        # [p, o, b]: xt[p, o, b] = x[b, o*128+p]
        xtc = np.ascontiguousarray(
            shard.T.reshape(32, 128, B_CORE).transpose(1, 0, 2)
        )
        in_maps.append({"xt": xtc, "AT": ATd, "R": Rd})

    res = run_bass_kernel_spmd(
        nc,
        in_maps,
        core_ids=list(range(N_CORES)),
        trace=trace,
        **(trace_kwargs or {}),
    )
    cidx = _col_gather_index()
    out = np.empty((BATCH, N), dtype=in_dtype)
    for c in range(N_CORES):
        y = np.asarray(res.results[c]["y"]).reshape(128 * 32, B_CORE)
        out[c * B_CORE:(c + 1) * B_CORE] = y[cidx].T
    return out, res


def kernel(x, twiddle):
    out, _ = run(x, twiddle)
    return out


# revision 7
# speedup vs baseline: 1.1585x; 1.1585x over previous
"""Butterfly multiply (n=4096, 12 stages, increasing stride) on 8 Trainium2
NeuronCores.

Math: the 12 butterfly stages factor into two dense matmul passes:
stages 0..6 (strides 1..64) compose into 32 dense 128x128 matrices A_o acting
within 128-aligned blocks; stages 7..11 (strides 128..2048) compose into 128
dense 32x32 matrices C_i acting across blocks at fixed within-block index.
Both are composed on the host from the tiny twiddle input.

Device pipeline per core (batch shard 1024 rows, fp16 compute):
  load x^T  ->  pass A (AT_o stationary)  ->  PSUM copy -> y1
  -> permute DMA  z[(a,o), (tau,b)] = y_mid[b, o*128+4tau+a]  (SBUF->SBUF)
  -> pass B (R_tau stationary, z moving)  -> psB [(o',a)-part, b]  (n-major)
  -> PSUM copy -> outT -> store n-major; host unscrambles columns.

The permute is SBUF read-port limited: each per-tau DMA reads 4 source
partitions.  Since pass A's output partition is a free column permutation of
the host-built AT matrices, y1 row (tau,a) is placed at partition
16*m(tau) + 4a + r(tau) so the 4 reads of one tau-DMA hit 4 DIFFERENT SBUF
AXI ports, and the tau->(m,r) map is built so the sync ring's taus (j in
{0,1} of each 4-tau wave) use even ports while the scalar ring's (j in {2,3})
use odd ports -- the two rings never collide on read ports.

DMA layout keeps descriptor runs big: xt dram [p, o, b] (load runs 32 KB),
y dram [v, tau, b] (store runs 8 KB), permute runs 2 KB (full 1024 batch).

Sharding: batch 8192 split across 8 cores (data parallel), twiddle-derived
matrices replicated.
"""

import os
import numpy as np

LOG_N = 12
N = 4096
BATCH = 8192
N_CORES = 8
B_CORE = BATCH // N_CORES  # 1024 rows per core

COMPUTE = os.environ.get("BUTTERFLY_COMPUTE", "fp16")
SPREAD = os.environ.get("BUTTERFLY_SPREAD", "1") == "1"
STORE_LAG = int(os.environ.get("BUTTERFLY_STORE_LAG", "2"))
# per-tau permute queue: g=gpsimd(SWDGE ~170GB/s), y=sync, s=scalar (HWDGE
# rings share one ~100GB/s descriptor generator, which also feeds stores)
PERM_PAT = os.environ.get(
    "BUTTERFLY_PERM_PAT",
    "ygggsggg" "yggggggg" "ygggsggg" "yggggggg",
)
OG = 4            # o's per load DMA group
SUB = 512         # matmul / PSUM-copy column granularity

def _tau_h(tau):
    """Base partition + stride of tau's 4 source rows (at h, h+st, h+2st, h+3st).
    Stride 8 within a 32-partition block spreads the 4 reads across 4 SBUF AXI
    ports under both plausible port wirings (p div 8, and the quad-interleave)."""
    if not SPREAD:
        return 4 * tau, 1  # contiguous quad, stride 1
    w, j = tau >> 2, tau & 3
    sloc = (w + 4 * (j & 1)) % 8
    return 32 * j + sloc, 8


def _i_of_p():
    """i_of_p[p] = which y_mid row (i' = 4 tau + a) pass A must emit at
    partition p.  Inverse of the placement in _tau_h."""
    if not SPREAD:
        return np.arange(128)
    iop = np.empty(128, dtype=np.int64)
    for p in range(128):
        j, rem = p >> 5, p & 31
        a, sloc = rem >> 3, rem & 7
        w = (sloc - 4 * (j & 1)) % 8
        iop[p] = 4 * (4 * w + j) + a
    assert len(set(iop.tolist())) == 128
    return iop


def _compose_matrices(twiddle):
    """Compose stages 0..6 -> A (32,128,128) and stages 7..11 -> C (128,32,32),
    in float64."""
    tw = np.asarray(twiddle)[0, 0].astype(np.float64)  # (12, 2048, 2, 2)

    A = np.zeros((32, 128, 128))
    A[:, np.arange(128), np.arange(128)] = 1.0
    for idx in range(7):
        s = 1 << idx
        Ar = A.reshape(32, 128 // (2 * s), 2, s, 128)  # (o, dl, k, j, i_in)
        o = np.arange(32)[:, None, None]
        dl = np.arange(128 // (2 * s))[None, :, None]
        j = np.arange(s)[None, None, :]
        m = (o * (64 // s) + dl) * s + j
        t = tw[idx, m]  # (32, dl, j, 2, 2)
        x0, x1 = Ar[:, :, 0], Ar[:, :, 1]
        new0 = t[..., 0, 0:1] * x0 + t[..., 0, 1:2] * x1
        new1 = t[..., 1, 0:1] * x0 + t[..., 1, 1:2] * x1
        A = np.stack([new0, new1], axis=2).reshape(32, 128, 128)

    C = np.zeros((128, 32, 32))
    C[:, np.arange(32), np.arange(32)] = 1.0
    for idx in range(7, 12):
        s = 1 << idx
        sp = s // 128
        Cr = C.reshape(128, 32 // (2 * sp), 2, sp, 32)  # (i, dl, k, ol, o_in)
        i = np.arange(128)[None, None, :]
        dl = np.arange(32 // (2 * sp))[:, None, None]
        ol = np.arange(sp)[None, :, None]
        m = dl * (128 * sp) + 128 * ol + i  # (dl, ol, i)
        t = np.moveaxis(tw[idx, m], 2, 0)  # (i, dl, ol, 2, 2)
        x0, x1 = Cr[:, :, 0], Cr[:, :, 1]
        new0 = t[..., 0, 0:1] * x0 + t[..., 0, 1:2] * x1
        new1 = t[..., 1, 0:1] * x0 + t[..., 1, 1:2] * x1
        C = np.stack([new0, new1], axis=2).reshape(128, 32, 32)

    # AT'[o][k, p] = A_o[i_of_p(p), k]   (sigma-permuted lhsT for pass A)
    AT = np.ascontiguousarray(np.transpose(A[:, _i_of_p(), :], (0, 2, 1)))
    # R[tau][u=(a*32+o_in), v=(o_out*4+a)] = C[4*tau+a][o_out, o_in]
    R = np.zeros((32, 128, 128))
    for tau in range(32):
        for a in range(4):
            R[tau, a * 32:(a + 1) * 32, a::4] = C[4 * tau + a].T
    return AT, R


def _build_program(np_dt, mybir_dt):
    """Trace + compile the per-core Bass program. Returns nc."""
    import concourse.bacc as bacc
    import concourse.tile as tile
    import concourse.mybir as mybir
    from contextlib import ExitStack

    f32 = mybir.dt.float32
    dt = mybir_dt
    B = B_CORE

    nc = bacc.Bacc(
        "TRN2",
        target_bir_lowering=False,
        debug=False,
        enable_asserts=False,
        num_devices=1,
    )
    # x shipped pre-transposed, partition-major: xt[p, o, b] = x[b, o*128+p]
    x_ap = nc.dram_tensor("xt", (128, 32, B), dt, kind="ExternalInput").ap()
    at_ap = nc.dram_tensor("AT", (128, 32 * 128), dt, kind="ExternalInput").ap()
    r_ap = nc.dram_tensor("R", (128, 32 * 128), dt, kind="ExternalInput").ap()
    # n-major output: y[v, tau, b]; host maps (tau, v) -> n
    y_ap = nc.dram_tensor("y", (128, 32, B), dt, kind="ExternalOutput").ap()

    with tile.TileContext(nc) as tc, ExitStack() as ctx:
        wpool = ctx.enter_context(tc.tile_pool(name="weights", bufs=1))
        xT_pool = ctx.enter_context(tc.tile_pool(name="xT", bufs=1))
        z_pool = ctx.enter_context(tc.tile_pool(name="z", bufs=1))
        out_pool = ctx.enter_context(tc.tile_pool(name="outT", bufs=2))
        psA_pool = ctx.enter_context(tc.tile_pool(name="psA", bufs=2, space="PSUM"))
        psB_pool = ctx.enter_context(tc.tile_pool(name="psB", bufs=2, space="PSUM"))

        ATw = wpool.tile([128, 32 * 128], dt, tag="ATw")
        Rw = wpool.tile([128, 32 * 128], dt, tag="Rw")
        nc.sync.dma_start(ATw[:], at_ap)
        nc.scalar.dma_start(Rw[:], r_ap)

        xT = xT_pool.tile([128, 32 * B], dt, tag="xT")
        y1 = xT  # pass A writes back over the x block it just consumed
        z = z_pool.tile([128, 32 * B], dt, tag="z")

        # 1. loads on sync ring (runs 32 KB -> big descriptors)
        for og in range(32 // OG):
            eng = nc.sync if og % 2 == 0 else nc.scalar
            eng.dma_start(
                xT[:, og * OG * B:(og + 1) * OG * B],
                x_ap[:, og * OG:(og + 1) * OG, :],
            )

        # 2. pass A; copies rotate vector/gpsimd/scalar (all idle here)
        def _copy(eng, dst, src):
            (eng.copy if eng is nc.scalar else eng.tensor_copy)(dst, src)

        cps = [nc.vector, nc.scalar]
        for o in range(32):
            psA = psA_pool.tile([128, B], f32, tag="psA")
            for ss in range(B // SUB):
                nc.tensor.matmul(
                    psA[:, ss * SUB:(ss + 1) * SUB],
                    ATw[:, o * 128:(o + 1) * 128],
                    xT[:, o * B + ss * SUB:o * B + (ss + 1) * SUB],
                    start=True,
                    stop=True,
                )
            _copy(cps[o % 2], y1[:, o * B:(o + 1) * B], psA[:])

        # 3+4. permute waves + pass B + stores, wave = 4 consecutive taus.
        #   perm j in {0,1} -> sync ring (even read ports)
        #   perm j in {2,3} -> scalar ring (odd read ports)
        #   stores -> scalar ring, interleaved STORE_LAG waves behind perm
        out_tiles = {}

        ENG = {"g": nc.gpsimd, "y": nc.sync, "s": nc.scalar}

        def emit_perm(w):
            for j in range(4):
                tau = 4 * w + j
                h, st = _tau_h(tau)
                ENG[PERM_PAT[tau]].dma_start(
                    z[:, tau * B:(tau + 1) * B],
                    y1[h:h + 3 * st + 1:st, :].rearrange("a (o b) -> a o b", b=B),
                )

        def emit_passB(w):
            outT = out_pool.tile([128, 4 * B], dt, tag="outT")
            out_tiles[w] = outT
            cpb = [nc.vector, nc.scalar]
            for tt in range(4):
                tau = 4 * w + tt
                psB = psB_pool.tile([128, B], f32, tag="psB")
                for ss in range(B // SUB):
                    nc.tensor.matmul(
                        psB[:, ss * SUB:(ss + 1) * SUB],
                        Rw[:, tau * 128:(tau + 1) * 128],
                        z[:, tau * B + ss * SUB:tau * B + (ss + 1) * SUB],
                        start=True,
                        stop=True,
                    )
                _copy(cpb[tt % 2], outT[:, tt * B:(tt + 1) * B], psB[:])

        def emit_store(w):
            eng = nc.scalar if w % 2 == 0 else nc.sync
            eng.dma_start(
                y_ap[:, 4 * w:4 * (w + 1), :], out_tiles.pop(w)[:]
            )

        for w in range(8):
            emit_perm(w)
            emit_passB(w)
            if w >= STORE_LAG:
                emit_store(w - STORE_LAG)
        for w in range(8 - STORE_LAG, 8):
            emit_store(w)

    nc.compile()
    return nc


_CACHE = {}


def _get_program():
    import concourse.mybir as mybir

    key = COMPUTE
    if key not in _CACHE:
        if COMPUTE == "fp16":
            _CACHE[key] = (_build_program(np.float16, mybir.dt.float16), np.float16)
        else:
            _CACHE[key] = (_build_program(np.float32, mybir.dt.float32), np.float32)
    return _CACHE[key]


def _col_gather_index():
    """c_of_n[n] = v(n)*32 + tau(n) into the (v,tau)-flattened device output."""
    n = np.arange(N)
    i = n % 128
    o_out = n // 128
    tau = i // 4
    a = i % 4
    v = o_out * 4 + a
    return v * 32 + tau


def run(x, twiddle, trace=False, trace_kwargs=None):
    """Run the butterfly kernel on 8 cores. Returns (out, BassKernelResults)."""
    from concourse.bass_utils import run_bass_kernel_spmd

    nc, np_dt = _get_program()

    AT, R = _compose_matrices(twiddle)
    ATd = np.ascontiguousarray(AT.transpose(1, 0, 2).reshape(128, 32 * 128)).astype(np_dt)
    Rd = np.ascontiguousarray(R.transpose(1, 0, 2).reshape(128, 32 * 128)).astype(np_dt)

    x = np.asarray(x)
    in_dtype = x.dtype
    xd = x.astype(np_dt)

    in_maps = []
    for c in range(N_CORES):
        shard = xd[c * B_CORE:(c + 1) * B_CORE]
        # [p, o, b]: xt[p, o, b] = x[b, o*128+p]
        xtc = np.ascontiguousarray(
            shard.T.reshape(32, 128, B_CORE).transpose(1, 0, 2)
        )
        in_maps.append({"xt": xtc, "AT": ATd, "R": Rd})

    res = run_bass_kernel_spmd(
        nc,
        in_maps,
        core_ids=list(range(N_CORES)),
        trace=trace,
        **(trace_kwargs or {}),
    )
    cidx = _col_gather_index()
    out = np.empty((BATCH, N), dtype=in_dtype)
    for c in range(N_CORES):
        y = np.asarray(res.results[c]["y"]).reshape(128 * 32, B_CORE)
        out[c * B_CORE:(c + 1) * B_CORE] = y[cidx].T
    return out, res


def kernel(x, twiddle):
    out, _ = run(x, twiddle)
    return out


# revision 8
# speedup vs baseline: 1.2304x; 1.0621x over previous
"""Butterfly multiply (n=4096, 12 stages, increasing stride) on 8 Trainium2
NeuronCores.

Math: the 12 butterfly stages factor into two dense matmul passes:
stages 0..6 (strides 1..64) compose into 32 dense 128x128 matrices A_o acting
within 128-aligned blocks; stages 7..11 (strides 128..2048) compose into 128
dense 32x32 matrices C_i acting across blocks at fixed within-block index.
Both are composed on the host from the tiny twiddle input.

Device pipeline per core (batch shard 1024 rows, fp16 compute):
  load x^T  ->  pass A (AT_o stationary)  ->  PSUM copy -> y1
  -> permute DMA  z[(a,o), (tau,b)] = y_mid[b, o*128+4tau+a]  (SBUF->SBUF)
  -> pass B (R_tau stationary, z moving)  -> psB [(o',a)-part, b]  (n-major)
  -> PSUM copy -> outT -> store n-major; host unscrambles columns.

The permute is SBUF read-port limited: each per-tau DMA reads 4 source
partitions.  Since pass A's output partition is a free column permutation of
the host-built AT matrices, y1 row (tau,a) is placed at partition
16*m(tau) + 4a + r(tau) so the 4 reads of one tau-DMA hit 4 DIFFERENT SBUF
AXI ports, and the tau->(m,r) map is built so the sync ring's taus (j in
{0,1} of each 4-tau wave) use even ports while the scalar ring's (j in {2,3})
use odd ports -- the two rings never collide on read ports.

DMA layout keeps descriptor runs big: xt dram [p, o, b] (load runs 32 KB),
y dram [v, tau, b] (store runs 8 KB), permute runs 2 KB (full 1024 batch).

Sharding: batch 8192 split across 8 cores (data parallel), twiddle-derived
matrices replicated.
"""

import os
import numpy as np

LOG_N = 12
N = 4096
BATCH = 8192
N_CORES = 8
B_CORE = BATCH // N_CORES  # 1024 rows per core

COMPUTE = os.environ.get("BUTTERFLY_COMPUTE", "fp16")
SPREAD = os.environ.get("BUTTERFLY_SPREAD", "1") == "1"
STORE_LAG = int(os.environ.get("BUTTERFLY_STORE_LAG", "2"))
# per-tau permute queue: g=gpsimd(SWDGE ~170GB/s), y=sync, s=scalar (HWDGE
# rings share one ~100GB/s descriptor generator, which also feeds stores)
PERM_PAT = os.environ.get(
    "BUTTERFLY_PERM_PAT",
    "ygggsggg" "ygggsggg" "ygggsggg" "ygsggggg",
)
OG = 4            # o's per load DMA group
SUB = 512         # matmul / PSUM-copy column granularity

def _tau_h(tau):
    """Base partition + stride of tau's 4 source rows (at h, h+st, h+2st, h+3st).
    Stride 8 within a 32-partition block spreads the 4 reads across 4 SBUF AXI
    ports under both plausible port wirings (p div 8, and the quad-interleave)."""
    if not SPREAD:
        return 4 * tau, 1  # contiguous quad, stride 1
    w, j = tau >> 2, tau & 3
    sloc = (w + 4 * (j & 1)) % 8
    return 32 * j + sloc, 8


def _i_of_p():
    """i_of_p[p] = which y_mid row (i' = 4 tau + a) pass A must emit at
    partition p.  Inverse of the placement in _tau_h."""
    if not SPREAD:
        return np.arange(128)
    iop = np.empty(128, dtype=np.int64)
    for p in range(128):
        j, rem = p >> 5, p & 31
        a, sloc = rem >> 3, rem & 7
        w = (sloc - 4 * (j & 1)) % 8
        iop[p] = 4 * (4 * w + j) + a
    assert len(set(iop.tolist())) == 128
    return iop


def _compose_matrices(twiddle):
    """Compose stages 0..6 -> A (32,128,128) and stages 7..11 -> C (128,32,32),
    in float64."""
    tw = np.asarray(twiddle)[0, 0].astype(np.float64)  # (12, 2048, 2, 2)

    A = np.zeros((32, 128, 128))
    A[:, np.arange(128), np.arange(128)] = 1.0
    for idx in range(7):
        s = 1 << idx
        Ar = A.reshape(32, 128 // (2 * s), 2, s, 128)  # (o, dl, k, j, i_in)
        o = np.arange(32)[:, None, None]
        dl = np.arange(128 // (2 * s))[None, :, None]
        j = np.arange(s)[None, None, :]
        m = (o * (64 // s) + dl) * s + j
        t = tw[idx, m]  # (32, dl, j, 2, 2)
        x0, x1 = Ar[:, :, 0], Ar[:, :, 1]
        new0 = t[..., 0, 0:1] * x0 + t[..., 0, 1:2] * x1
        new1 = t[..., 1, 0:1] * x0 + t[..., 1, 1:2] * x1
        A = np.stack([new0, new1], axis=2).reshape(32, 128, 128)

    C = np.zeros((128, 32, 32))
    C[:, np.arange(32), np.arange(32)] = 1.0
    for idx in range(7, 12):
        s = 1 << idx
        sp = s // 128
        Cr = C.reshape(128, 32 // (2 * sp), 2, sp, 32)  # (i, dl, k, ol, o_in)
        i = np.arange(128)[None, None, :]
        dl = np.arange(32 // (2 * sp))[:, None, None]
        ol = np.arange(sp)[None, :, None]
        m = dl * (128 * sp) + 128 * ol + i  # (dl, ol, i)
        t = np.moveaxis(tw[idx, m], 2, 0)  # (i, dl, ol, 2, 2)
        x0, x1 = Cr[:, :, 0], Cr[:, :, 1]
        new0 = t[..., 0, 0:1] * x0 + t[..., 0, 1:2] * x1
        new1 = t[..., 1, 0:1] * x0 + t[..., 1, 1:2] * x1
        C = np.stack([new0, new1], axis=2).reshape(128, 32, 32)

    # AT'[o][k, p] = A_o[i_of_p(p), k]   (sigma-permuted lhsT for pass A)
    AT = np.ascontiguousarray(np.transpose(A[:, _i_of_p(), :], (0, 2, 1)))
    # R[tau][u=(a*32+o_in), v=(o_out*4+a)] = C[4*tau+a][o_out, o_in]
    R = np.zeros((32, 128, 128))
    for tau in range(32):
        for a in range(4):
            R[tau, a * 32:(a + 1) * 32, a::4] = C[4 * tau + a].T
    return AT, R


def _build_program(np_dt, mybir_dt):
    """Trace + compile the per-core Bass program. Returns nc."""
    import concourse.bacc as bacc
    import concourse.tile as tile
    import concourse.mybir as mybir
    from contextlib import ExitStack

    f32 = mybir.dt.float32
    dt = mybir_dt
    B = B_CORE

    nc = bacc.Bacc(
        "TRN2",
        target_bir_lowering=False,
        debug=False,
        enable_asserts=False,
        num_devices=1,
    )
    # x shipped pre-transposed, partition-major: xt[p, o, b] = x[b, o*128+p]
    x_ap = nc.dram_tensor("xt", (128, 32, B), dt, kind="ExternalInput").ap()
    at_ap = nc.dram_tensor("AT", (128, 32 * 128), dt, kind="ExternalInput").ap()
    r_ap = nc.dram_tensor("R", (128, 32 * 128), dt, kind="ExternalInput").ap()
    # n-major output: y[v, tau, b]; host maps (tau, v) -> n
    y_ap = nc.dram_tensor("y", (128, 32, B), dt, kind="ExternalOutput").ap()

    with tile.TileContext(nc) as tc, ExitStack() as ctx:
        wpool = ctx.enter_context(tc.tile_pool(name="weights", bufs=1))
        xT_pool = ctx.enter_context(tc.tile_pool(name="xT", bufs=1))
        z_pool = ctx.enter_context(tc.tile_pool(name="z", bufs=1))
        out_pool = ctx.enter_context(tc.tile_pool(name="outT", bufs=2))
        psA_pool = ctx.enter_context(tc.tile_pool(name="psA", bufs=4, space="PSUM"))
        psB_pool = ctx.enter_context(tc.tile_pool(name="psB", bufs=4, space="PSUM"))

        ATw = wpool.tile([128, 32 * 128], dt, tag="ATw")
        Rw = wpool.tile([128, 32 * 128], dt, tag="Rw")
        nc.sync.dma_start(ATw[:], at_ap)
        nc.scalar.dma_start(Rw[:], r_ap)

        xT = xT_pool.tile([128, 32 * B], dt, tag="xT")
        y1 = xT  # pass A writes back over the x block it just consumed
        z = z_pool.tile([128, 32 * B], dt, tag="z")

        # 1. loads on sync ring (runs 32 KB -> big descriptors)
        for og in range(32 // OG):
            eng = nc.sync if og % 2 == 0 else nc.scalar
            eng.dma_start(
                xT[:, og * OG * B:(og + 1) * OG * B],
                x_ap[:, og * OG:(og + 1) * OG, :],
            )

        # 2. pass A; copies rotate vector/gpsimd/scalar (all idle here)
        def _copy(eng, dst, src):
            (eng.copy if eng is nc.scalar else eng.tensor_copy)(dst, src)

        cps = [nc.vector, nc.scalar]
        ci = 0
        for o in range(32):
            for ss in range(B // SUB):
                psA = psA_pool.tile([128, SUB], f32, tag="psA")
                nc.tensor.matmul(
                    psA[:],
                    ATw[:, o * 128:(o + 1) * 128],
                    xT[:, o * B + ss * SUB:o * B + (ss + 1) * SUB],
                    start=True,
                    stop=True,
                )
                _copy(cps[ci % 2],
                      y1[:, o * B + ss * SUB:o * B + (ss + 1) * SUB], psA[:])
                ci += 1

        # 3+4. permute waves + pass B + stores, wave = 4 consecutive taus.
        #   perm j in {0,1} -> sync ring (even read ports)
        #   perm j in {2,3} -> scalar ring (odd read ports)
        #   stores -> scalar ring, interleaved STORE_LAG waves behind perm
        out_tiles = {}

        ENG = {"g": nc.gpsimd, "y": nc.sync, "s": nc.scalar}

        def emit_perm(w):
            for j in range(4):
                tau = 4 * w + j
                h, st = _tau_h(tau)
                ENG[PERM_PAT[tau]].dma_start(
                    z[:, tau * B:(tau + 1) * B],
                    y1[h:h + 3 * st + 1:st, :].rearrange("a (o b) -> a o b", b=B),
                )

        def emit_passB(w):
            outT = out_pool.tile([128, 4 * B], dt, tag="outT")
            out_tiles[w] = outT
            cpb = [nc.vector, nc.scalar]
            for tt in range(4):
                tau = 4 * w + tt
                for ss in range(B // SUB):
                    psB = psB_pool.tile([128, SUB], f32, tag="psB")
                    nc.tensor.matmul(
                        psB[:],
                        Rw[:, tau * 128:(tau + 1) * 128],
                        z[:, tau * B + ss * SUB:tau * B + (ss + 1) * SUB],
                        start=True,
                        stop=True,
                    )
                    _copy(cpb[(2 * tt + ss) % 2],
                          outT[:, tt * B + ss * SUB:tt * B + (ss + 1) * SUB],
                          psB[:])

        def emit_store(w):
            eng = nc.scalar if w % 2 == 0 else nc.sync
            eng.dma_start(
                y_ap[:, 4 * w:4 * (w + 1), :], out_tiles.pop(w)[:]
            )

        for w in range(8):
            emit_perm(w)
            emit_passB(w)
            if w >= STORE_LAG:
                emit_store(w - STORE_LAG)
        for w in range(8 - STORE_LAG, 8):
            emit_store(w)

    nc.compile()
    return nc


_CACHE = {}


def _get_program():
    import concourse.mybir as mybir

    key = COMPUTE
    if key not in _CACHE:
        if COMPUTE == "fp16":
            _CACHE[key] = (_build_program(np.float16, mybir.dt.float16), np.float16)
        else:
            _CACHE[key] = (_build_program(np.float32, mybir.dt.float32), np.float32)
    return _CACHE[key]


def _col_gather_index():
    """c_of_n[n] = v(n)*32 + tau(n) into the (v,tau)-flattened device output."""
    n = np.arange(N)
    i = n % 128
    o_out = n // 128
    tau = i // 4
    a = i % 4
    v = o_out * 4 + a
    return v * 32 + tau


def run(x, twiddle, trace=False, trace_kwargs=None):
    """Run the butterfly kernel on 8 cores. Returns (out, BassKernelResults)."""
    from concourse.bass_utils import run_bass_kernel_spmd

    nc, np_dt = _get_program()

    AT, R = _compose_matrices(twiddle)
    ATd = np.ascontiguousarray(AT.transpose(1, 0, 2).reshape(128, 32 * 128)).astype(np_dt)
    Rd = np.ascontiguousarray(R.transpose(1, 0, 2).reshape(128, 32 * 128)).astype(np_dt)

    x = np.asarray(x)
    in_dtype = x.dtype
    xd = x.astype(np_dt)

    in_maps = []
    for c in range(N_CORES):
        shard = xd[c * B_CORE:(c + 1) * B_CORE]
        # [p, o, b]: xt[p, o, b] = x[b, o*128+p]
        xtc = np.ascontiguousarray(
            shard.T.reshape(32, 128, B_CORE).transpose(1, 0, 2)
        )
        in_maps.append({"xt": xtc, "AT": ATd, "R": Rd})

    res = run_bass_kernel_spmd(
        nc,
        in_maps,
        core_ids=list(range(N_CORES)),
        trace=trace,
        **(trace_kwargs or {}),
    )
    cidx = _col_gather_index()
    out = np.empty((BATCH, N), dtype=in_dtype)
    for c in range(N_CORES):
        y = np.asarray(res.results[c]["y"]).reshape(128 * 32, B_CORE)
        out[c * B_CORE:(c + 1) * B_CORE] = y[cidx].T
    return out, res


def kernel(x, twiddle):
    out, _ = run(x, twiddle)
    return out
